# revision 14
# baseline (speedup 1.0000x reference)
"""Multi-head attention Trainium2 kernel (8-core SPMD).

Problem: N=4096 locations, d_model=512, H=4 heads, d_k=128, d_v=256.
  q = Q@Wq[h]; k = K@Wk[h]; v = V@Wv[h]
  scores = q k^T / sqrt(N); weights = softmax(scores)
  out = concat_h(weights @ v) @ Wo^T

Sharding: hybrid sequence/projection parallel.
  - Queries: core c owns rows [512c, 512c+512) end-to-end (scores,
    softmax, AV, output projection); outputs are disjoint row blocks
    that the host concatenates.
  - K/V projections: core c computes kT[h] and v[h] only for ITS
    512-key slice, then a per-head AllGather distributes the full
    kT/v to every core (4 small AGs, pipelined against attention).

The host pre-arranges inputs into SBUF layouts (transposes are a host
layout prep; fp32 DMA transpose does not exist on this HW). Matmul
operands are bf16 (full PE rate, fast weight load); accumulation is
always fp32 in PSUM; softmax statistics are exact fp32.

Per-core dataflow:
  qT[h]  = Wq[h]^T QT_slice          [d_k, 512]       (16 MMs)
  kT_c[h]= Wk[h]^T KT_cslice         [d_k, 512]       (16 MMs)
  v_c[h] = VT_cslice^T Wv[h-pair]    [512, 2, 256]    (32 MMs)
  AllGather_h {kT_c[h], v_c[h]}  ->  full kT[h], v[h] in HBM
  per 512-key superchunk kc (DMA from gathered buffer):
    per 128-key slice:
      scoresT = kT_slice^T qT[h]     [128, 512] psum
      E       = exp(scoresT/64)      ACT -> bf16
      heads[qs] += E_slice^T v_aug   [128, 258] psum (ones col = rowsum)
  normalize by rowsum, PE-transpose to headsT[dv, q],
  out = sum_j headsT_j^T WoT_j       [512, 512].
"""

import sys

if '/opt/trn_rl_repo' not in sys.path:
    sys.path.insert(0, '/opt/trn_rl_repo')

import numpy as np

import concourse.bass as bass
import concourse.tile as tile
from concourse import mybir
from concourse import bass_utils
from concourse.masks import make_identity

N = 4096
D = 512
H = 4
DK = 128
DV = 256
N_CORES = 8
QR = N // N_CORES          # query rows per core
KS = N // N_CORES          # key rows per core (projection shard)
KC = N // 512              # 512-key superchunks
F32 = mybir.dt.float32
F32R = mybir.dt.float32r
BF16 = mybir.dt.bfloat16
EXP = mybir.ActivationFunctionType.Exp

KHALF = 128 * 512          # elems of the kT part of one AG block
VPART = 128 * DV           # elems of one 128-key v slice
BLK = KHALF + 4 * VPART    # per-rank AG block per head


def split_multi_waits(nc, max_waits=1):
    """This container's walrus accepts only 1 sync-wait per instruction;
    move excess waits onto preceding same-engine Drain instructions."""
    for fn in nc.m.functions:
        for blk in fn.blocks:
            insts = list(blk.instructions)
            new, n_split = [], 0
            for inst in insts:
                si = getattr(inst, 'sync_info', None)
                ow = list(si.on_wait) if si is not None and si.on_wait else []
                if len(ow) > max_waits:
                    excess, keep = ow[:-max_waits], ow[-max_waits:]
                    si.on_wait = keep
                    for j, w in enumerate(excess):
                        new.append(mybir.InstDrain(
                            name=f"{inst.name}-ws{j}", engine=inst.engine,
                            ins=[], outs=[],
                            sync_info=mybir.SyncInfo(on_wait=[w], on_update=[]),
                        ))
                        n_split += 1
                new.append(inst)
            if n_split:
                blk.instructions = new
    return nc


def build_nc():
    nc = bass.Bass("TRN2", target_bir_lowering=False, debug=False,
                   num_devices=N_CORES)
    # all inputs pre-arranged by the host into SBUF layouts
    QTs = nc.dram_tensor("qts", [128, 4, QR], BF16, kind="ExternalInput").ap()
    KTC = nc.dram_tensor("ktc", [128, 4, KS], BF16, kind="ExternalInput").ap()
    VTC = nc.dram_tensor("vtc", [128, 4, KS], BF16, kind="ExternalInput").ap()
    WQ = nc.dram_tensor("wq", [128, H, 4, DK], BF16, kind="ExternalInput").ap()
    WK = nc.dram_tensor("wk", [128, H, 4, DK], BF16, kind="ExternalInput").ap()
    WV = nc.dram_tensor("wv", [128, 2, 4, 2, DV], BF16, kind="ExternalInput").ap()
    WOT = nc.dram_tensor("wot", [128, 2 * H, D], F32R, kind="ExternalInput").ap()
    OUT = nc.dram_tensor("out", [QR, D], F32, kind="ExternalOutput").ap()
    cci = [nc.dram_tensor(f"cci{h}", [BLK], BF16).ap() for h in range(H)]
    cco = [nc.dram_tensor(f"cco{h}", [N_CORES, BLK], BF16,
                          addr_space="Shared").ap() for h in range(H)]

    with tile.TileContext(nc) as tc:
        with tc.tile_pool(name="const", bufs=1) as const, \
             tc.tile_pool(name="ktd", bufs=3) as ktdp, \
             tc.tile_pool(name="vaug", bufs=4) as vaugp, \
             tc.tile_pool(name="esb", bufs=4) as esbp, \
             tc.tile_pool(name="bnc", bufs=3) as bncp, \
             tc.tile_pool(name="outsb", bufs=2) as outp, \
             tc.tile_pool(name="psh", bufs=4, space="PSUM") as ps_heads, \
             tc.tile_pool(name="pssc", bufs=3, space="PSUM") as ps_sc:
            # ---- resident tensors --------------------------------------
            wq_sb = const.tile([128, H, 4, DK], BF16)
            wk_sb = const.tile([128, H, 4, DK], BF16)
            wv_sb = const.tile([128, 2, 4, 2, DV], BF16)
            wot_sb = const.tile([128, 2 * H, D], F32R)
            ktc_sb = const.tile([128, 4, KS], BF16)
            vtc_sb = const.tile([128, 4, KS], BF16)
            ident = const.tile([128, 128], F32)
            make_identity(nc, ident[:])
            rec = const.tile([128, H * 4], F32)
            qt_sb = const.tile([128, H, QR], BF16)
            headsT = const.tile([128, 2 * H, QR], F32R)

            qts = ktdp.tile([128, 4, QR], BF16, tag="ktd")
            nc.sync.dma_start(qts[:], QTs[:])
            nc.sync.dma_start(wq_sb[:], WQ[:])
            nc.sync.dma_start(wk_sb[:], WK[:])
            nc.sync.dma_start(ktc_sb[:], KTC[:])
            nc.sync.dma_start(vtc_sb[:], VTC[:])
            nc.sync.dma_start(wv_sb[:], WV[:])

            # ---- q projections -----------------------------------------
            for h in range(H):
                qp = ps_sc.tile([128, QR], F32, tag="sc")
                for c in range(4):
                    nc.tensor.matmul(qp[:], wq_sb[:, h, c, :], qts[:, c, :],
                                     start=(c == 0), stop=(c == 3))
                nc.vector.tensor_copy(qt_sb[:, h, :], qp[:])

            # ---- local kT / v projection shards, AG per head asap ------
            for e in range(2):
                for h in (2 * e, 2 * e + 1):
                    ktp = ps_sc.tile([128, KS], F32, tag="sc", name="ktp")
                    for c in range(4):
                        nc.tensor.matmul(ktp[:], wk_sb[:, h, c, :],
                                         ktc_sb[:, c, :],
                                         start=(c == 0), stop=(c == 3))
                    kb = bncp.tile([128, KS], BF16, tag="bk")
                    nc.vector.tensor_copy(kb[:], ktp[:])
                    nc.sync.dma_start(
                        cci[h][0:KHALF].rearrange("(p f) -> p f", p=128),
                        kb[:])
                for ks in range(4):
                    vp = ps_sc.tile([128, 2, DV], F32, tag="sc", name="vp")
                    for c in range(4):
                        nc.tensor.matmul(
                            vp[:], vtc_sb[:, c, 128 * ks:128 * (ks + 1)],
                            wv_sb[:, e, c, :, :],
                            start=(c == 0), stop=(c == 3))
                    vb = bncp.tile([128, 2, DV], BF16, tag="bv")
                    nc.vector.tensor_copy(vb[:], vp[:])
                    for f in range(2):
                        nc.sync.dma_start(
                            cci[2 * e + f]
                            [KHALF + VPART * ks:KHALF + VPART * (ks + 1)]
                            .rearrange("(p f) -> p f", p=128),
                            vb[:, f, :])
                for h in (2 * e, 2 * e + 1):
                    nc.gpsimd.collective_compute(
                        "AllGather", mybir.AluOpType.bypass,
                        replica_groups=[list(range(N_CORES))],
                        ins=[cci[h][:]], outs=[cco[h][:]])

            # ---- attention, head by head -------------------------------
            for h in range(H):
                hp = [ps_heads.tile([128, DV + 2], F32, tag="heads",
                                    name=f"hp{h}_{i}")
                      for i in range(4)]

                def emit_av(p):
                    e_t, v_t, kg_t = p
                    for qs in range(4):
                        nc.tensor.matmul(
                            hp[qs][:], e_t[:, 128 * qs:128 * (qs + 1)],
                            v_t[:],
                            start=(kg_t == 0), stop=(kg_t == 4 * KC - 1),
                            skip_group_check=True)

                pending = []
                for kc in range(KC):
                    ktd = ktdp.tile([128, 512], BF16, tag="ktd", name="ktd")
                    nc.sync.dma_start(
                        ktd[:],
                        cco[h][kc, 0:KHALF].rearrange("(p f) -> p f", p=128))
                    for ks in range(4):
                        kg = 4 * kc + ks          # global 128-key slice
                        vaug = vaugp.tile([128, DV + 2], BF16)
                        nc.sync.dma_start(
                            vaug[:, 0:DV],
                            cco[h][kc, KHALF + VPART * ks:
                                   KHALF + VPART * (ks + 1)]
                            .rearrange("(p f) -> p f", p=128))
                        nc.vector.memset(vaug[:, DV:DV + 2], 1.0)

                        sp = ps_sc.tile([128, QR], F32, tag="sc")
                        nc.tensor.matmul(sp[:], ktd[:, 128 * ks:128 * (ks + 1)],
                                         qt_sb[:, h, :], start=True, stop=True)
                        esb = esbp.tile([128, QR], BF16)
                        nc.scalar.activation(esb[:], sp[:], EXP, scale=1.0 / 64.0)

                        pending.append((esb, vaug, kg))
                        if len(pending) > 2:
                            emit_av(pending.pop(0))
                for p in pending:
                    emit_av(p)
                pending = []

                # normalize + transpose heads -> headsT[dv, q]
                for qs in range(4):
                    r = rec[:, 4 * h + qs:4 * h + qs + 1]
                    nc.vector.reciprocal(r, hp[qs][:, DV:DV + 1])
                    hn = outp.tile([128, DV], F32, tag="out")
                    nc.vector.tensor_scalar_mul(hn[:], hp[qs][:, 0:DV], r)
                    for half in range(2):
                        tp = ps_sc.tile([128, 512], F32, tag="sc")
                        nc.tensor.transpose(tp[:, 0:128],
                                            hn[:, 128 * half:128 * (half + 1)],
                                            ident[:])
                        nc.vector.tensor_copy(
                            headsT[:, 2 * h + half, 128 * qs:128 * (qs + 1)],
                            tp[:, 0:128])

            # ---- output projection -------------------------------------
            nc.sync.dma_start(wot_sb[:], WOT[:])
            for qs in range(4):
                op = ps_sc.tile([128, 512], F32, tag="sc")
                for j in range(2 * H):
                    nc.tensor.matmul(op[:], headsT[:, j, 128 * qs:128 * (qs + 1)],
                                     wot_sb[:, j, :],
                                     start=(j == 0), stop=(j == 2 * H - 1))
                osb = outp.tile([128, D], F32, tag="out")
                nc.vector.tensor_copy(osb[:], op[:])
                nc.sync.dma_start(OUT[128 * qs:128 * (qs + 1), :], osb[:])

    return split_multi_waits(nc)


_NC_CACHE = []


def _get_nc():
    if not _NC_CACHE:
        _NC_CACHE.append(build_nc())
    return _NC_CACHE[0]


def _in_maps(Q, K, V, Wq, Wk, Wv, Wo):
    import ml_dtypes
    f = np.float32
    bf = ml_dtypes.bfloat16

    def to_pcn(xT):
        # [D, n] -> [128, 4, n] with row d = 128*c + p
        return np.ascontiguousarray(
            xT.reshape(4, 128, xT.shape[1]).transpose(1, 0, 2))

    QT = np.asarray(Q, dtype=f).T.astype(bf)           # [D, N]
    KTr = to_pcn(np.asarray(K, dtype=f).T.astype(bf))  # [128, 4, N]
    VTr = to_pcn(np.asarray(V, dtype=f).T.astype(bf))
    # Wq/Wk [h, D, dk] -> [128, h, c, dk]
    Wqr = np.ascontiguousarray(
        np.asarray(Wq, dtype=f).astype(bf)
        .reshape(H, 4, 128, DK).transpose(2, 0, 1, 3))
    Wkr = np.ascontiguousarray(
        np.asarray(Wk, dtype=f).astype(bf)
        .reshape(H, 4, 128, DK).transpose(2, 0, 1, 3))
    # Wv [h=2e+f, D, dv] -> [128, e, c, f, dv]
    Wvr = np.ascontiguousarray(
        np.asarray(Wv, dtype=f).astype(bf)
        .reshape(2, 2, 4, 128, DV).transpose(3, 0, 2, 1, 4))
    # Wo [D, H*DV] -> WoT [H*DV, D] -> [128, j, D]
    WOTr = np.ascontiguousarray(
        np.asarray(Wo, dtype=f).T.reshape(2 * H, 128, D).transpose(1, 0, 2))
    maps = []
    for c in range(N_CORES):
        qts = np.ascontiguousarray(
            QT[:, QR * c:QR * (c + 1)].reshape(4, 128, QR).transpose(1, 0, 2))
        maps.append({
            "qts": qts,
            "ktc": np.ascontiguousarray(KTr[:, :, KS * c:KS * (c + 1)]),
            "vtc": np.ascontiguousarray(VTr[:, :, KS * c:KS * (c + 1)]),
            "wq": Wqr, "wk": Wkr, "wv": Wvr, "wot": WOTr,
        })
    return maps


def run(inputs, trace=False, trace_cores=None):
    """Run the SPMD kernel; returns (full_output, BassKernelResults)."""
    nc = _get_nc()
    maps = _in_maps(**inputs)
    res = bass_utils.run_bass_kernel_spmd(
        nc, maps, core_ids=list(range(N_CORES)),
        trace=trace, trace_cores=trace_cores)
    out = np.concatenate([res.results[c]["out"] for c in range(N_CORES)], axis=0)
    return out, res


def kernel(**inputs) -> np.ndarray:
    out, _ = run(inputs)
    return out


# revision 15
# speedup vs baseline: 1.3171x; 1.3171x over previous
"""Multi-head attention Trainium2 kernel (8-core SPMD, sequence-parallel).

Problem: N=4096 locations, d_model=512, H=4 heads, d_k=128, d_v=256.
  q = Q@Wq[h]; k = K@Wk[h]; v = V@Wv[h]
  scores = q k^T / sqrt(N); weights = softmax(scores)
  out = concat_h(weights @ v) @ Wo^T

Sharding: core c owns query rows [512c, 512c+512). K/V work is computed
per-core (fully local, no collectives). Host passes transposed layouts
(QT/KT/VT/WoT) so every matmul contracts along the partition axis with
natural SBUF tiles; all matmul operands are float32r (full PE rate,
~2e-4 relative error).

Per-core dataflow (all heads):
  qT[h]  = Wq[h]^T QT_slice          [d_k, 512]
  per 512-key superchunk (KT streamed from HBM):
    kT   = Wk[h]^T KT_chunk          [d_k, 512]
    per 128-key slice:
      v      = VT_slice^T Wv[h]      [128, 256] (+ ones col -> 257)
      scoresT= kT_slice^T qT[h]      [128 keys, 512 q] (psum)
      E      = exp(scoresT / 64)     (ACT, psum -> sbuf f32r)
      heads[qs] += E_slice^T v_aug   [128 q, 257] accumulated in psum
  normalize heads by the ones-column rowsum, transpose via PE to
  headsT[dv, q], then out = sum_j headsT_j^T WoT_j -> [512, 512].
"""

import sys

if '/opt/trn_rl_repo' not in sys.path:
    sys.path.insert(0, '/opt/trn_rl_repo')

import numpy as np

import concourse.bass as bass
import concourse.tile as tile
from concourse import mybir
from concourse import bass_utils
from concourse.masks import make_identity

N = 4096
D = 512
H = 4
DK = 128
DV = 256
N_CORES = 8
QR = N // N_CORES          # query rows per core
KC = N // 512              # 512-key superchunks
F32 = mybir.dt.float32
F32R = mybir.dt.float32r
BF16 = mybir.dt.bfloat16
EXP = mybir.ActivationFunctionType.Exp


def split_multi_waits(nc, max_waits=1):
    """This container's walrus accepts only 1 sync-wait per instruction;
    move excess waits onto preceding same-engine Drain instructions."""
    for fn in nc.m.functions:
        for blk in fn.blocks:
            insts = list(blk.instructions)
            new, n_split = [], 0
            for inst in insts:
                si = getattr(inst, 'sync_info', None)
                ow = list(si.on_wait) if si is not None and si.on_wait else []
                if len(ow) > max_waits:
                    excess, keep = ow[:-max_waits], ow[-max_waits:]
                    si.on_wait = keep
                    for j, w in enumerate(excess):
                        new.append(mybir.InstDrain(
                            name=f"{inst.name}-ws{j}", engine=inst.engine,
                            ins=[], outs=[],
                            sync_info=mybir.SyncInfo(on_wait=[w], on_update=[]),
                        ))
                        n_split += 1
                new.append(inst)
            if n_split:
                blk.instructions = new
    return nc


def build_nc():
    nc = bass.Bass("TRN2", target_bir_lowering=False, debug=False,
                   num_devices=N_CORES)
    # all inputs pre-arranged by the host into SBUF layouts
    QTs = nc.dram_tensor("qts", [128, 4, QR], BF16, kind="ExternalInput").ap()
    KT = nc.dram_tensor("kt", [128, 4, N], BF16, kind="ExternalInput").ap()
    VT = nc.dram_tensor("vt", [128, 4, N], BF16, kind="ExternalInput").ap()
    WQ = nc.dram_tensor("wq", [128, H, 4, DK], BF16, kind="ExternalInput").ap()
    WK = nc.dram_tensor("wk", [128, H, 4, DK], BF16, kind="ExternalInput").ap()
    WV = nc.dram_tensor("wv", [128, 2, 4, 2, DV], BF16, kind="ExternalInput").ap()
    WOT = nc.dram_tensor("wot", [128, 2 * H, D], F32R, kind="ExternalInput").ap()
    OUT = nc.dram_tensor("out", [QR, D], F32, kind="ExternalOutput").ap()

    with tile.TileContext(nc) as tc:
        with tc.tile_pool(name="const", bufs=1) as const, \
             tc.tile_pool(name="ktd", bufs=4) as ktdp, \
             tc.tile_pool(name="ktsb", bufs=3) as ktsbp, \
             tc.tile_pool(name="esb", bufs=4) as esbp, \
             tc.tile_pool(name="outsb", bufs=2) as outp, \
             tc.tile_pool(name="psh", bufs=4, space="PSUM") as ps_heads, \
             tc.tile_pool(name="pskv", bufs=2, space="PSUM") as ps_kv, \
             tc.tile_pool(name="pssc", bufs=2, space="PSUM") as ps_sc:
            # ---- resident tensors (DMAs emitted in first-use order) ----
            wq_sb = const.tile([128, H, 4, DK], BF16)
            wk_sb = const.tile([128, H, 4, DK], BF16)
            # v weights for head-pair batching: [pair][c][h-in-pair][dv]
            wv_sb = const.tile([128, 2, 4, 2, DV], BF16)
            wot_sb = const.tile([128, 2 * H, D], F32R)
            vt_sb = const.tile([128, 4, N], BF16)
            vstore = const.tile([128, N // 128, DV], BF16)
            ident = const.tile([128, 128], F32)
            make_identity(nc, ident[:])
            vaug_slots = [const.tile([128, DV + 2], BF16, name=f"vaug{i}")
                          for i in range(4)]
            for i in range(4):
                nc.vector.memset(vaug_slots[i][:, DV:DV + 2], 1.0)
            rec = const.tile([128, H * 4], F32)
            qt_sb = const.tile([128, H, QR], BF16)
            headsT = const.tile([128, 2 * H, QR], F32R)

            # ---- q projections ----------------------------------------
            qts = ktdp.tile([128, 4, 512], BF16, tag="ktd")
            nc.sync.dma_start(qts[:], QTs[:])
            nc.sync.dma_start(wq_sb[:], WQ[:])
            nc.sync.dma_start(wk_sb[:], WK[:])
            nc.sync.dma_start(wv_sb[:], WV[:])
            for h in range(H):
                qp = ps_sc.tile([128, QR], F32, tag="sc")
                for c in range(4):
                    nc.tensor.matmul(qp[:], wq_sb[:, h, c, :], qts[:, c, :],
                                     start=(c == 0), stop=(c == 3))
                nc.vector.tensor_copy(qt_sb[:, h, :], qp[:])

            # ---- attention, head by head ------------------------------
            for h in range(H):
                hp = [ps_heads.tile([128, DV + 2], F32, tag="heads",
                                    name=f"hp{h}_{i}")
                      for i in range(4)]
                def emit_av(p):
                    e_t, v_t, kg_t = p
                    for qs in range(4):
                        nc.tensor.matmul(
                            hp[qs][:], e_t[:, 128 * qs:128 * (qs + 1)],
                            v_t[:],
                            start=(kg_t == 0), stop=(kg_t == 4 * KC - 1),
                            skip_group_check=True)

                pending = []
                for kc in range(KC):
                    if h == 0:
                        nc.sync.dma_start(
                            vt_sb[:, :, 512 * kc:512 * (kc + 1)],
                            VT[:, :, 512 * kc:512 * (kc + 1)])
                    ktd = ktdp.tile([128, 4, 512], BF16, tag="ktd")
                    nc.sync.dma_start(
                        ktd[:], KT[:, :, 512 * kc:512 * (kc + 1)])
                    ktp = ps_kv.tile([128, 512], F32, tag="kv")
                    for c in range(4):
                        nc.tensor.matmul(ktp[:], wk_sb[:, h, c, :], ktd[:, c, :],
                                         start=(c == 0), stop=(c == 3))
                    kt_sb = ktsbp.tile([128, 512], BF16)
                    nc.vector.tensor_copy(kt_sb[:], ktp[:])

                    for ks in range(4):
                        kg = 4 * kc + ks          # global 128-key slice
                        vaug = vaug_slots[kg % 4]
                        if h % 2 == 0:
                            vp = ps_kv.tile([128, 2, DV], F32, tag="kv",
                                            name="vp")
                            for c in range(4):
                                nc.tensor.matmul(
                                    vp[:],
                                    vt_sb[:, c, 128 * kg:128 * (kg + 1)],
                                    wv_sb[:, h // 2, c, :, :],
                                    start=(c == 0), stop=(c == 3))
                            nc.vector.tensor_copy(vaug[:, 0:DV], vp[:, 0, :])
                            nc.vector.tensor_copy(vstore[:, kg, :], vp[:, 1, :])
                        else:
                            nc.vector.tensor_copy(vaug[:, 0:DV], vstore[:, kg, :])

                        sp = ps_sc.tile([128, QR], F32, tag="sc")
                        nc.tensor.matmul(sp[:], kt_sb[:, 128 * ks:128 * (ks + 1)],
                                         qt_sb[:, h, :], start=True, stop=True)
                        esb = esbp.tile([128, QR], BF16)
                        nc.scalar.activation(esb[:], sp[:], EXP, scale=1.0 / 64.0)

                        pending.append((esb, vaug, kg))
                        if len(pending) > 2:
                            emit_av(pending.pop(0))
                for p in pending:
                    emit_av(p)
                pending = []

                # normalize + transpose heads -> headsT[dv, q]
                for qs in range(4):
                    r = rec[:, 4 * h + qs:4 * h + qs + 1]
                    nc.vector.reciprocal(r, hp[qs][:, DV:DV + 1])
                    hn = outp.tile([128, DV], F32, tag="out")
                    nc.vector.tensor_scalar_mul(hn[:], hp[qs][:, 0:DV], r)
                    for half in range(2):
                        tp = ps_sc.tile([128, 512], F32, tag="sc")
                        nc.tensor.transpose(tp[:, 0:128],
                                            hn[:, 128 * half:128 * (half + 1)],
                                            ident[:])
                        nc.vector.tensor_copy(
                            headsT[:, 2 * h + half, 128 * qs:128 * (qs + 1)],
                            tp[:, 0:128])

            # ---- output projection ------------------------------------
            nc.sync.dma_start(wot_sb[:], WOT[:])
            for qs in range(4):
                op = ps_sc.tile([128, 512], F32, tag="sc")
                for j in range(2 * H):
                    nc.tensor.matmul(op[:], headsT[:, j, 128 * qs:128 * (qs + 1)],
                                     wot_sb[:, j, :],
                                     start=(j == 0), stop=(j == 2 * H - 1))
                osb = outp.tile([128, D], F32, tag="out")
                nc.vector.tensor_copy(osb[:], op[:])
                nc.sync.dma_start(OUT[128 * qs:128 * (qs + 1), :], osb[:])

    return split_multi_waits(nc)


_NC_CACHE = []


def _get_nc():
    if not _NC_CACHE:
        _NC_CACHE.append(build_nc())
    return _NC_CACHE[0]


def _in_maps(Q, K, V, Wq, Wk, Wv, Wo):
    import ml_dtypes
    f = np.float32
    bf = ml_dtypes.bfloat16

    def to_pcn(xT):
        # [D, n] -> [128, 4, n] with row d = 128*c + p
        return np.ascontiguousarray(
            xT.reshape(4, 128, xT.shape[1]).transpose(1, 0, 2))

    QT = np.asarray(Q, dtype=f).T.astype(bf)          # [D, N]
    KTr = to_pcn(np.asarray(K, dtype=f).T.astype(bf))
    VTr = to_pcn(np.asarray(V, dtype=f).T.astype(bf))
    # Wq/Wk [h, D, dk] -> [128, h, c, dk]
    Wqr = np.ascontiguousarray(
        np.asarray(Wq, dtype=f).astype(bf)
        .reshape(H, 4, 128, DK).transpose(2, 0, 1, 3))
    Wkr = np.ascontiguousarray(
        np.asarray(Wk, dtype=f).astype(bf)
        .reshape(H, 4, 128, DK).transpose(2, 0, 1, 3))
    # Wv [h=2e+f, D, dv] -> [128, e, c, f, dv]
    Wvr = np.ascontiguousarray(
        np.asarray(Wv, dtype=f).astype(bf)
        .reshape(2, 2, 4, 128, DV).transpose(3, 0, 2, 1, 4))
    # Wo [D, H*DV] -> WoT [H*DV, D] -> [128, j, D]
    WOTr = np.ascontiguousarray(
        np.asarray(Wo, dtype=f).T.reshape(2 * H, 128, D).transpose(1, 0, 2))
    maps = []
    for c in range(N_CORES):
        qts = np.ascontiguousarray(
            QT[:, QR * c:QR * (c + 1)].reshape(4, 128, QR).transpose(1, 0, 2))
        maps.append({
            "qts": qts, "kt": KTr, "vt": VTr,
            "wq": Wqr, "wk": Wkr, "wv": Wvr, "wot": WOTr,
        })
    return maps


def run(inputs, trace=False, trace_cores=None):
    """Run the SPMD kernel; returns (full_output, BassKernelResults)."""
    nc = _get_nc()
    maps = _in_maps(**inputs)
    res = bass_utils.run_bass_kernel_spmd(
        nc, maps, core_ids=list(range(N_CORES)),
        trace=trace, trace_cores=trace_cores)
    out = np.concatenate([res.results[c]["out"] for c in range(N_CORES)], axis=0)
    return out, res


def kernel(**inputs) -> np.ndarray:
    out, _ = run(inputs)
    return out


# revision 21
# speedup vs baseline: 1.3277x; 1.0080x over previous
"""Multi-head attention Trainium2 kernel (8-core SPMD, sequence-parallel).

Problem: N=4096 locations, d_model=512, H=4 heads, d_k=128, d_v=256.
  q = Q@Wq[h]; k = K@Wk[h]; v = V@Wv[h]
  scores = q k^T / sqrt(N); weights = softmax(scores)
  out = concat_h(weights @ v) @ Wo^T

Sharding: core c owns query rows [512c, 512c+512). K/V work is computed
per-core (fully local, no collectives). Host passes transposed layouts
(QT/KT/VT/WoT) so every matmul contracts along the partition axis with
natural SBUF tiles; all matmul operands are float32r (full PE rate,
~2e-4 relative error).

Per-core dataflow (all heads):
  qT[h]  = Wq[h]^T QT_slice          [d_k, 512]
  per 512-key superchunk (KT streamed from HBM):
    kT   = Wk[h]^T KT_chunk          [d_k, 512]
    per 128-key slice:
      v      = VT_slice^T Wv[h]      [128, 256] (+ ones col -> 257)
      scoresT= kT_slice^T qT[h]      [128 keys, 512 q] (psum)
      E      = exp(scoresT / 64)     (ACT, psum -> sbuf f32r)
      heads[qs] += E_slice^T v_aug   [128 q, 257] accumulated in psum
  normalize heads by the ones-column rowsum, transpose via PE to
  headsT[dv, q], then out = sum_j headsT_j^T WoT_j -> [512, 512].
"""

import sys

if '/opt/trn_rl_repo' not in sys.path:
    sys.path.insert(0, '/opt/trn_rl_repo')

import numpy as np

import concourse.bass as bass
import concourse.tile as tile
from concourse import mybir
from concourse import bass_utils
from concourse.masks import make_identity

N = 4096
D = 512
H = 4
DK = 128
DV = 256
N_CORES = 8
QR = N // N_CORES          # query rows per core
KC = N // 512              # 512-key superchunks
F32 = mybir.dt.float32
F32R = mybir.dt.float32r
BF16 = mybir.dt.bfloat16
EXP = mybir.ActivationFunctionType.Exp


def split_multi_waits(nc, max_waits=1):
    """This container's walrus accepts only 1 sync-wait per instruction;
    move excess waits onto preceding same-engine Drain instructions."""
    for fn in nc.m.functions:
        for blk in fn.blocks:
            insts = list(blk.instructions)
            new, n_split = [], 0
            for inst in insts:
                si = getattr(inst, 'sync_info', None)
                ow = list(si.on_wait) if si is not None and si.on_wait else []
                if len(ow) > max_waits:
                    excess, keep = ow[:-max_waits], ow[-max_waits:]
                    si.on_wait = keep
                    for j, w in enumerate(excess):
                        new.append(mybir.InstDrain(
                            name=f"{inst.name}-ws{j}", engine=inst.engine,
                            ins=[], outs=[],
                            sync_info=mybir.SyncInfo(on_wait=[w], on_update=[]),
                        ))
                        n_split += 1
                new.append(inst)
            if n_split:
                blk.instructions = new
    return nc


def build_nc():
    nc = bass.Bass("TRN2", target_bir_lowering=False, debug=False,
                   num_devices=N_CORES)
    # all inputs pre-arranged by the host into SBUF layouts
    QTs = nc.dram_tensor("qts", [128, 4, QR], BF16, kind="ExternalInput").ap()
    KT = nc.dram_tensor("kt", [128, 4, N], BF16, kind="ExternalInput").ap()
    VT = nc.dram_tensor("vt", [128, 4, N], BF16, kind="ExternalInput").ap()
    WQ = nc.dram_tensor("wq", [128, H, 4, DK], BF16, kind="ExternalInput").ap()
    WK = nc.dram_tensor("wk", [128, H, 4, DK], BF16, kind="ExternalInput").ap()
    WV = nc.dram_tensor("wv", [128, 2, 4, 2, DV], BF16, kind="ExternalInput").ap()
    WOT = nc.dram_tensor("wot", [128, 2 * H, D], F32R, kind="ExternalInput").ap()
    OUT = nc.dram_tensor("out", [QR, D], F32, kind="ExternalOutput").ap()

    with tile.TileContext(nc) as tc:
        with tc.tile_pool(name="const", bufs=1) as const, \
             tc.tile_pool(name="ktd", bufs=4) as ktdp, \
             tc.tile_pool(name="ktsb", bufs=3) as ktsbp, \
             tc.tile_pool(name="esb", bufs=4) as esbp, \
             tc.tile_pool(name="hn", bufs=5) as hnp, \
             tc.tile_pool(name="outsb", bufs=2) as outp, \
             tc.tile_pool(name="psh", bufs=4, space="PSUM") as ps_heads, \
             tc.tile_pool(name="pskv", bufs=2, space="PSUM") as ps_kv, \
             tc.tile_pool(name="pssc", bufs=2, space="PSUM") as ps_sc:
            # ---- resident tensors (DMAs emitted in first-use order) ----
            wq_sb = const.tile([128, H, 4, DK], BF16)
            wk_sb = const.tile([128, H, 4, DK], BF16)
            # v weights for head-pair batching: [pair][c][h-in-pair][dv]
            wv_sb = const.tile([128, 2, 4, 2, DV], BF16)
            wot_sb = const.tile([128, 2 * H, D], F32R)
            vt_sb = const.tile([128, 4, N], BF16)
            vstore = const.tile([128, N // 128, DV], BF16)
            ident = const.tile([128, 128], BF16)
            make_identity(nc, ident[:])
            vaug_slots = [const.tile([128, DV + 2], BF16, name=f"vaug{i}")
                          for i in range(4)]
            for i in range(4):
                nc.vector.memset(vaug_slots[i][:, DV:DV + 2], 1.0)
            rec = const.tile([128, H * 4], F32)
            qt_sb = const.tile([128, H, QR], BF16)
            headsT = const.tile([128, 2 * H, QR], F32R)

            # ---- q projections ----------------------------------------
            qts = ktdp.tile([128, 4, 512], BF16, tag="ktd")
            nc.sync.dma_start(qts[:], QTs[:])
            nc.sync.dma_start(wq_sb[:], WQ[:])
            nc.sync.dma_start(wk_sb[:], WK[:])
            nc.sync.dma_start(wv_sb[:], WV[:])
            for h in range(H):
                qp = ps_sc.tile([128, QR], F32, tag="sc")
                for c in range(4):
                    nc.tensor.matmul(qp[:], wq_sb[:, h, c, :], qts[:, c, :],
                                     start=(c == 0), stop=(c == 3))
                nc.vector.tensor_copy(qt_sb[:, h, :], qp[:])

            # ---- attention, head by head ------------------------------
            deferred_tr = []

            def emit_transposes():
                while deferred_tr:
                    hh, qs, hn = deferred_tr.pop(0)
                    for half in range(2):
                        tp = ps_sc.tile([128, 1024], BF16, tag="sc", name="tp")
                        nc.tensor.transpose(tp[:, 0:128],
                                            hn[:, 128 * half:128 * (half + 1)],
                                            ident[:])
                        nc.vector.tensor_copy(
                            headsT[:, 2 * hh + half, 128 * qs:128 * (qs + 1)],
                            tp[:, 0:128])

            for h in range(H):
                hp = [ps_heads.tile([128, DV + 2], F32, tag="heads",
                                    name=f"hp{h}_{i}")
                      for i in range(4)]
                def emit_av(p):
                    e_t, v_t, kg_t = p
                    for qs in range(4):
                        nc.tensor.matmul(
                            hp[qs][:], e_t[:, 128 * qs:128 * (qs + 1)],
                            v_t[:],
                            start=(kg_t == 0), stop=(kg_t == 4 * KC - 1),
                            skip_group_check=True)

                pending = []
                for kc in range(KC):
                    if kc == 2:
                        emit_transposes()
                    if h == 0:
                        nc.sync.dma_start(
                            vt_sb[:, :, 512 * kc:512 * (kc + 1)],
                            VT[:, :, 512 * kc:512 * (kc + 1)])
                    ktd = ktdp.tile([128, 4, 512], BF16, tag="ktd")
                    nc.sync.dma_start(
                        ktd[:], KT[:, :, 512 * kc:512 * (kc + 1)])
                    ktp = ps_kv.tile([128, 512], F32, tag="kv")
                    for c in range(4):
                        nc.tensor.matmul(ktp[:], wk_sb[:, h, c, :], ktd[:, c, :],
                                         start=(c == 0), stop=(c == 3))
                    kt_sb = ktsbp.tile([128, 512], BF16)
                    nc.vector.tensor_copy(kt_sb[:], ktp[:])

                    for ks in range(4):
                        kg = 4 * kc + ks          # global 128-key slice
                        vaug = vaug_slots[kg % 4]
                        if h % 2 == 0:
                            vp = ps_kv.tile([128, 2, DV], F32, tag="kv",
                                            name="vp")
                            for c in range(4):
                                nc.tensor.matmul(
                                    vp[:],
                                    vt_sb[:, c, 128 * kg:128 * (kg + 1)],
                                    wv_sb[:, h // 2, c, :, :],
                                    start=(c == 0), stop=(c == 3))
                            nc.vector.tensor_copy(vaug[:, 0:DV], vp[:, 0, :])
                            nc.vector.tensor_copy(vstore[:, kg, :], vp[:, 1, :])
                        else:
                            nc.vector.tensor_copy(vaug[:, 0:DV], vstore[:, kg, :])

                        sp = ps_sc.tile([128, QR], F32, tag="sc")
                        nc.tensor.matmul(sp[:], kt_sb[:, 128 * ks:128 * (ks + 1)],
                                         qt_sb[:, h, :], start=True, stop=True)
                        esb = esbp.tile([128, QR], BF16)
                        nc.scalar.activation(esb[:], sp[:], EXP, scale=1.0 / 64.0)

                        pending.append((esb, vaug, kg))
                        if len(pending) > 2:
                            emit_av(pending.pop(0))
                for p in pending:
                    emit_av(p)
                pending = []

                # normalize now (frees heads psum); transposes deferred
                # into the next head's steady state
                for qs in range(4):
                    r = rec[:, 4 * h + qs:4 * h + qs + 1]
                    nc.vector.reciprocal(r, hp[qs][:, DV:DV + 1])
                    hn = hnp.tile([128, DV], BF16, tag="hn")
                    nc.vector.tensor_scalar_mul(hn[:], hp[qs][:, 0:DV], r)
                    deferred_tr.append((h, qs, hn))
            emit_transposes()

            # ---- output projection ------------------------------------
            nc.sync.dma_start(wot_sb[:], WOT[:])
            for qs in range(4):
                op = ps_sc.tile([128, 512], F32, tag="sc")
                for j in range(2 * H):
                    nc.tensor.matmul(op[:], headsT[:, j, 128 * qs:128 * (qs + 1)],
                                     wot_sb[:, j, :],
                                     start=(j == 0), stop=(j == 2 * H - 1))
                osb = outp.tile([128, D], F32, tag="out")
                nc.vector.tensor_copy(osb[:], op[:])
                nc.sync.dma_start(OUT[128 * qs:128 * (qs + 1), :], osb[:])

    return split_multi_waits(nc)


_NC_CACHE = []


def _get_nc():
    if not _NC_CACHE:
        _NC_CACHE.append(build_nc())
    return _NC_CACHE[0]


def _in_maps(Q, K, V, Wq, Wk, Wv, Wo):
    import ml_dtypes
    f = np.float32
    bf = ml_dtypes.bfloat16

    def to_pcn(xT):
        # [D, n] -> [128, 4, n] with row d = 128*c + p
        return np.ascontiguousarray(
            xT.reshape(4, 128, xT.shape[1]).transpose(1, 0, 2))

    QT = np.asarray(Q, dtype=f).T.astype(bf)          # [D, N]
    KTr = to_pcn(np.asarray(K, dtype=f).T.astype(bf))
    VTr = to_pcn(np.asarray(V, dtype=f).T.astype(bf))
    # Wq/Wk [h, D, dk] -> [128, h, c, dk]
    Wqr = np.ascontiguousarray(
        np.asarray(Wq, dtype=f).astype(bf)
        .reshape(H, 4, 128, DK).transpose(2, 0, 1, 3))
    Wkr = np.ascontiguousarray(
        np.asarray(Wk, dtype=f).astype(bf)
        .reshape(H, 4, 128, DK).transpose(2, 0, 1, 3))
    # Wv [h=2e+f, D, dv] -> [128, e, c, f, dv]
    Wvr = np.ascontiguousarray(
        np.asarray(Wv, dtype=f).astype(bf)
        .reshape(2, 2, 4, 128, DV).transpose(3, 0, 2, 1, 4))
    # Wo [D, H*DV] -> WoT [H*DV, D] -> [128, j, D]
    WOTr = np.ascontiguousarray(
        np.asarray(Wo, dtype=f).T.reshape(2 * H, 128, D).transpose(1, 0, 2))
    maps = []
    for c in range(N_CORES):
        qts = np.ascontiguousarray(
            QT[:, QR * c:QR * (c + 1)].reshape(4, 128, QR).transpose(1, 0, 2))
        maps.append({
            "qts": qts, "kt": KTr, "vt": VTr,
            "wq": Wqr, "wk": Wkr, "wv": Wvr, "wot": WOTr,
        })
    return maps


def run(inputs, trace=False, trace_cores=None):
    """Run the SPMD kernel; returns (full_output, BassKernelResults)."""
    nc = _get_nc()
    maps = _in_maps(**inputs)
    res = bass_utils.run_bass_kernel_spmd(
        nc, maps, core_ids=list(range(N_CORES)),
        trace=trace, trace_cores=trace_cores)
    out = np.concatenate([res.results[c]["out"] for c in range(N_CORES)], axis=0)
    return out, res


def kernel(**inputs) -> np.ndarray:
    out, _ = run(inputs)
    return out
